# revision 13
# baseline (speedup 1.0000x reference)
"""Causal multi-head attention on 8 trn2 NeuronCores.

Problem (hardcoded): x [4, 2048, 2048] fp32, W_qkv [6144, 2048], W_out
[2048, 2048];  y = OutProj(CausalMHA(QKV(x))),  16 heads x 128.

Sharding: data-parallel over batch (4) x tensor-parallel over heads (2
groups of 8 heads).  Core c handles batch c//2, head-group c%2.  Each
core computes a partial output y_partial = attn_out_g @ W_out_g^T; the
host sums the two TP partials per batch.

v4: all matmuls in bf16 (HW-measured 331 ns/MM at N=512 vs fp32r's
462).  PSUM accumulation fp32.  x^T fully resident in SBUF (64KB bf16)
so phase 1 is single-pass.  All PSUM drains and the causal mask run on
DVE (ACT only does exp; gpsimd only partition-broadcast) — those were
the bf16-slow engines that sank the naive bf16 port.

  phase 1: QKV projection.  V: x chunk stationary vs 512 W_v columns.
           K,Q: weight chunk stationary vs all four 512-wide t-groups,
           accumulated in two paired [128,1024] PSUM tiles.
  phase 2: per head, two passes over query-group pairs (qg 0,1 / 2,3).
           Per k-chunk kc (2-deep software pipeline): scores for the
           active qgs into one paired PSUM tile; ONE exp on ACT over
           the pair (bf16 out); causal mask via DVE multiply with
           precomputed 0/1 tiles; PV and denominator (ones, M=1)
           accumulate per-qg in PSUM; normalize on DVE.
  phase 3: out-proj: attn + W_out resident (one 64KB tile), W_out
           chunk stationary vs all four t-groups, 4 PSUM accumulators.
"""

import numpy as np

D = 2048
T = 2048
B = 4
DH = 128
HPC = 8            # heads per core
SCALE = DH ** -0.5
VEG = 512          # V e-group width in phase 1
LAG = 2            # kc-granule software pipeline depth in phase 2

_compiled = None   # cached nc so repeated kernel() calls skip rebuild


def _build(loop_k=None, phases=(1, 2, 3)):
    import concourse.bacc as bacc_mod
    import concourse.mybir as mybir
    import concourse.tile as tile

    fp32 = mybir.dt.float32
    bf16 = mybir.dt.bfloat16

    nc = bacc_mod.Bacc(None, target_bir_lowering=False, debug=False)
    with tile.TileContext(nc) as tc:
        with tc.tile_pool(name="dram", bufs=1, space="DRAM") as dram:
            x_t = dram.tile([D, T], bf16, kind="ExternalInput", name="x_t",
                            uniquify=False)
            wqk = dram.tile([16, 128, 16, 128], bf16, kind="ExternalInput",
                            name="wqk", uniquify=False)
            wv = dram.tile([1024 // VEG, 128, 16, VEG], bf16,
                           kind="ExternalInput", name="wv", uniquify=False)
            wout = dram.tile([128, 8, D], bf16, kind="ExternalInput",
                             name="wout", uniquify=False)
            y_t = dram.tile([D, T], fp32, kind="ExternalOutput", name="y_t",
                            uniquify=False)
            qk_stage = dram.tile([2048, T], bf16, name="qk_stage")
            v_stage = dram.tile([HPC, 128, 16, 128], bf16, name="v_stage")
            attn_stage = dram.tile([1024, T], bf16, name="attn_stage")

            import contextlib
            loop_cm = (tc.For_i(0, loop_k, 1) if loop_k
                       else contextlib.nullcontext())
            with loop_cm:
                _emit_body(nc, tc, x_t, wqk, wv, wout, y_t, qk_stage,
                           v_stage, attn_stage, mybir, phases)
    nc.compile()
    return nc


def _emit_body(nc, tc, x_t, wqk, wv, wout, y_t, qk_stage, v_stage,
               attn_stage, mybir, phases=(1, 2, 3)):
    fp32 = mybir.dt.float32
    bf16 = mybir.dt.bfloat16
    Act = mybir.ActivationFunctionType
    Alu = mybir.AluOpType

    with (
        tc.tile_pool(name="big", bufs=1) as big,
        tc.tile_pool(name="wload", bufs=3) as wload,
        tc.tile_pool(name="outc", bufs=3) as outc,
        tc.tile_pool(name="exp", bufs=LAG + 1) as expp,
        tc.tile_pool(name="misc", bufs=2) as misc,
        tc.tile_pool(name="psp", bufs=1, space="PSUM") as psp,
    ):
        if 1 in phases:
            # ---------------- phase 1: QKV projection ----------------
            xt_sb = big.tile([128, 16, T], bf16, tag="big", name="xt_sb")
            nc.sync.dma_start(
                xt_sb[:],
                x_t[:].rearrange("(ko ki) t -> ki ko t", ki=128))

            # V: x chunk stationary, 512 W_v cols streamed per MM.
            # v_stage[h] = [ki(t), ko(t), dh]
            for eh in range(1024 // VEG):
                wv_sb = wload.tile([128, 16, VEG], bf16, tag="wv16",
                                   name="wv_sb")
                nc.sync.dma_start(wv_sb[:], wv[eh])
                for tt in range(16):
                    ps = psp.tile([128, VEG], fp32, tag="pv", bufs=2,
                                  name="ps_v")
                    for ko in range(16):
                        nc.tensor.matmul(
                            ps[:],
                            xt_sb[:, ko, tt * 128:(tt + 1) * 128],
                            wv_sb[:, ko],
                            start=(ko == 0), stop=(ko == 15))
                    ot = outc.tile([128, VEG], bf16, tag="out",
                                   name="ot_v")
                    nc.vector.tensor_copy(ot[:], ps[:])
                    for sub in range(VEG // 128):
                        nc.sync.dma_start(
                            v_stage[4 * eh + sub, :, tt, :],
                            ot[:, sub * 128:(sub + 1) * 128])

            # K and Q per head: qk_stage[e, t] (rows 0..1024 = K
            # head-major, 1024..2048 = Q head-major).  Weight chunk
            # stationary, streamed vs all four 512-wide t-groups into
            # two paired [128,1024] PSUM tiles.
            for h in range(HPC):
                for et in (h, 8 + h):
                    wq_sb = wload.tile([128, 16, 128], bf16, tag="wqk",
                                       name="wq_sb")
                    nc.sync.dma_start(wq_sb[:], wqk[et])
                    sp01 = psp.tile([128, 1024], fp32, tag="sp", bufs=2,
                                    name="ps_qk01")
                    sp23 = psp.tile([128, 1024], fp32, tag="sp", bufs=2,
                                    name="ps_qk23")
                    halves = [sp01[:, 0:512], sp01[:, 512:1024],
                              sp23[:, 0:512], sp23[:, 512:1024]]
                    for ko in range(16):
                        for tg in range(4):
                            nc.tensor.matmul(
                                halves[tg], wq_sb[:, ko],
                                xt_sb[:, ko, tg * 512:(tg + 1) * 512],
                                start=(ko == 0), stop=(ko == 15))
                    ot = outc.tile([128, 2048], bf16, tag="out",
                                   name="ot_qk")
                    nc.vector.tensor_copy(ot[:, 0:1024], sp01[:])
                    nc.vector.tensor_copy(ot[:, 1024:2048], sp23[:])
                    nc.sync.dma_start(
                        qk_stage[et * 128:(et + 1) * 128, :], ot[:])

        if 2 in phases:
            # ---------------- phase 2: attention per head ----------------
            ones_b = misc.tile([128, 1], bf16, tag="ones_b")
            nc.vector.memset(ones_b[:], 1.0)
            # 4 causal 0/1 mask tiles: mask_r keeps (qq >= kk + 128*r)
            masks = []
            for r in range(4):
                mk = misc.tile([128, 512], bf16, tag=f"mask{r}", bufs=1,
                               name=f"mask{r}")
                nc.vector.memset(mk[:], 1.0)
                nc.gpsimd.affine_select(
                    out=mk[:], in_=mk[:], compare_op=Alu.is_ge, fill=0.0,
                    base=-128 * r, channel_multiplier=-1,
                    pattern=[[1, 512]])
                masks.append(mk)

            for h in range(HPC):
                # packed per-head [k | q | v] tile: kqv[:, 0:4] = K^T
                # [dh, t], [:, 4:8] = Q^T [dh, t], [:, 8:12] = V in
                # [ki(t), ko(t), dh] chunk-major order.
                kqv = wload.tile([128, 12, 512], bf16, tag="wv16",
                                 name="kqv")
                nc.sync.dma_start(
                    kqv[:, 0:4, :],
                    qk_stage[h * 128:(h + 1) * 128]
                    .rearrange("p (a b) -> p a b", b=512))
                nc.sync.dma_start(
                    kqv[:, 4:8, :],
                    qk_stage[1024 + h * 128:1024 + (h + 1) * 128]
                    .rearrange("p (a b) -> p a b", b=512))
                nc.sync.dma_start(
                    kqv[:, 8:12, :],
                    v_stage[h].rearrange("p ko dh -> p (ko dh)")
                    .rearrange("p (a b) -> p a b", b=512))

                def kt_chunk(kc):
                    return kqv[:, kc // 4, (kc % 4) * 128:(kc % 4 + 1) * 128]

                def qt_slice(qg):
                    return kqv[:, 4 + qg, :]

                def vt_chunk(kc):
                    return kqv[:, 8 + kc // 4,
                               (kc % 4) * 128:(kc % 4 + 1) * 128]

                for qgs in ([0, 1], [2, 3]):
                    kc_max = 4 * (qgs[-1] + 1)
                    ps_o = {}
                    ps_se = {}
                    for qg in qgs:
                        ps_o[qg] = psp.tile([128, 512], fp32, tag="pv",
                                            bufs=2, name=f"ps_o{qg}")
                        ps_se[qg] = psp.tile([1, 512], fp32, tag="se",
                                             bufs=2, name=f"ps_se{qg}")
                    sp_tiles = [None] * kc_max
                    ex_tiles = [None] * kc_max

                    def active(kc, qgs=qgs):
                        return [qg for qg in qgs if kc < 4 * (qg + 1)]

                    def s_group(kc, qgs=qgs, sp_tiles=sp_tiles):
                        sp = psp.tile([128, 1024], fp32, tag="sp", bufs=2,
                                      name="ps_s")
                        sp_tiles[kc] = sp
                        for j, qg in enumerate(qgs):
                            if kc < 4 * (qg + 1):
                                nc.tensor.matmul(
                                    sp[:, j * 512:(j + 1) * 512],
                                    kt_chunk(kc), qt_slice(qg),
                                    start=True, stop=True)

                    def post_group(kc, qgs=qgs, sp_tiles=sp_tiles,
                                   ex_tiles=ex_tiles):
                        ex = expp.tile([128, 1024], bf16, tag="ex",
                                       name="ex")
                        ex_tiles[kc] = ex
                        acts = active(kc)
                        lo = (1 if len(acts) == 1 else 0) * 512
                        nc.scalar.activation(
                            ex[:, lo:1024], sp_tiles[kc][:, lo:1024],
                            Act.Exp, scale=SCALE)
                        for j, qg in enumerate(qgs):
                            r = kc - 4 * qg
                            if 0 <= r < 4:   # diagonal chunk: causal mask
                                w = 128 * (r + 1)
                                s0 = j * 512
                                nc.vector.tensor_mul(
                                    out=ex[:, s0:s0 + w],
                                    in0=ex[:, s0:s0 + w],
                                    in1=masks[r][:, 0:w])

                    def pv_group(kc, qgs=qgs, ex_tiles=ex_tiles):
                        acts = active(kc)
                        for j, qg in enumerate(qgs):
                            if qg in acts:
                                nc.tensor.matmul(
                                    ps_o[qg][:], vt_chunk(kc),
                                    ex_tiles[kc][:, j * 512:(j + 1) * 512],
                                    start=(kc == 0),
                                    stop=(kc == 4 * (qg + 1) - 1))
                        for j, qg in enumerate(qgs):
                            if qg in acts:
                                nc.tensor.matmul(
                                    ps_se[qg][:], ones_b[:],
                                    ex_tiles[kc][:, j * 512:(j + 1) * 512],
                                    start=(kc == 0),
                                    stop=(kc == 4 * (qg + 1) - 1))

                    for kc in range(kc_max):
                        s_group(kc)
                        if kc >= 1:
                            post_group(kc - 1)
                        if kc >= LAG:
                            pv_group(kc - LAG)
                    post_group(kc_max - 1)
                    for j in range(max(0, kc_max - LAG), kc_max):
                        pv_group(j)

                    for qg in qgs:
                        recip = misc.tile([1, 512], fp32, tag="recip",
                                          bufs=2, name="recip")
                        nc.vector.reciprocal(recip[:], ps_se[qg][:])
                        bc = misc.tile([128, 512], fp32, tag="bc", bufs=2,
                                       name="bc")
                        nc.gpsimd.partition_broadcast(bc[:], recip[:])
                        nsb = misc.tile([128, 512], bf16, tag="nsb",
                                        bufs=2, name="nsb")
                        nc.vector.tensor_mul(out=nsb[:], in0=ps_o[qg][:],
                                             in1=bc[:])
                        nc.sync.dma_start(
                            attn_stage[h * 128:(h + 1) * 128,
                                       qg * 512:(qg + 1) * 512], nsb[:])

        if 3 in phases:
            # ---------------- phase 3: output projection (bf16) ----------
            wa = big.tile([128, 16, D], bf16, tag="big", name="wa_sb")
            wout_sb = wa[:, 0:8]
            at_sb = wa[:, 8:16]
            nc.sync.dma_start(wout_sb, wout[:])
            nc.sync.dma_start(
                at_sb,
                attn_stage[:].rearrange("(fo fi) t -> fi fo t", fi=128))
            for et in range(D // 128):
                sp = psp.tile([128, 1024], fp32, tag="sp", bufs=2,
                              name="ps_y01")
                pv0 = psp.tile([128, 512], fp32, tag="pv", bufs=2,
                               name="ps_y2")
                pv1 = psp.tile([128, 512], fp32, tag="pv", bufs=2,
                               name="ps_y3")
                accs = [sp[:, 0:512], sp[:, 512:1024], pv0[:], pv1[:]]
                for fo in range(8):
                    for tg in range(4):
                        nc.tensor.matmul(
                            accs[tg],
                            wout_sb[:, fo, et * 128:(et + 1) * 128],
                            at_sb[:, fo, tg * 512:(tg + 1) * 512],
                            start=(fo == 0), stop=(fo == 7))
                ot = outc.tile([128, 1024], fp32, tag="out", name="ot_y01")
                nc.scalar.copy(ot[:], sp[:])
                nc.sync.dma_start(
                    y_t[et * 128:(et + 1) * 128, 0:1024], ot[:])
                ot2 = outc.tile([128, 1024], fp32, tag="out", name="ot_y23")
                nc.scalar.copy(ot2[:, 0:512], pv0[:])
                nc.scalar.copy(ot2[:, 512:1024], pv1[:])
                nc.sync.dma_start(
                    y_t[et * 128:(et + 1) * 128, 1024:2048], ot2[:])


def get_nc():
    global _compiled
    if _compiled is None:
        _compiled = _build()
    return _compiled


def make_in_maps(x, W_qkv, W_out):
    """Host-side sharding: per-core input dict (8 cores), bf16."""
    import ml_dtypes
    bf16 = np.dtype(ml_dtypes.bfloat16)
    x = np.asarray(x, dtype=np.float32)
    W_qkv = np.asarray(W_qkv, dtype=np.float32)
    W_out = np.asarray(W_out, dtype=np.float32)
    in_maps = []
    for c in range(8):
        b, g = divmod(c, 2)
        gs = slice(g * 1024, (g + 1) * 1024)
        Wq_g = W_qkv[0 * D:1 * D][gs]          # [1024, 2048]
        Wk_g = W_qkv[1 * D:2 * D][gs]
        Wv_g = W_qkv[2 * D:3 * D][gs]
        E_cat = np.concatenate([Wk_g, Wq_g], 0)  # rows: K then Q
        in_maps.append({
            "x_t": np.ascontiguousarray(x[b].T).astype(bf16),
            "wqk": np.ascontiguousarray(
                E_cat.reshape(16, 128, 16, 128)
                .transpose(0, 3, 2, 1)).astype(bf16),
            "wv": np.ascontiguousarray(
                Wv_g.reshape(1024 // VEG, VEG, 16, 128)
                .transpose(0, 3, 2, 1)).astype(bf16),
            "wout": np.ascontiguousarray(
                W_out[:, gs].T.reshape(8, 128, D)
                .transpose(1, 0, 2)).astype(bf16),
        })
    return in_maps


def combine_outputs(results):
    """results: list of 8 per-core dicts with 'y_t' -> full y [B, T, D]."""
    y = np.empty((B, T, D), dtype=np.float32)
    for b in range(B):
        y[b] = (results[2 * b]["y_t"] + results[2 * b + 1]["y_t"]).T
    return y


def kernel(x, W_qkv, W_out):
    from concourse.bass_utils import run_bass_kernel_spmd

    nc = get_nc()
    in_maps = make_in_maps(x, W_qkv, W_out)
    res = run_bass_kernel_spmd(nc, in_maps, core_ids=list(range(8)))
    return combine_outputs(res.results)
